# revision 10
# baseline (speedup 1.0000x reference)
"""Trainium2 Bass kernel for nn_BaselineMoEModel_71425306132874 (moe_routing).

Model: x:[B=4, S=1024, D=512] through blocks [MoE0, TFB0, MoE1, TFB1], then
final LN -> mean over tokens -> [512,10] classifier head.

Sharding: 4096 tokens over 8 cores as (batch = core//2, half = core%2), 512
tokens/core. All compute is token-parallel except attention, which needs the
full 1024-token sequence of the core's batch: each core computes K^T
(feature-major) and V (token-major) for its 512 tokens and swaps them with
its pair peer via an AllGather over groups [[0,1],[2,3],[4,5],[6,7]]. The
program is rank-symmetric: peer-slot selection is data-driven via a host-fed
one-hot `peer_sel` input, so one SPMD NEFF serves all 8 cores. Own-half
attention runs from SBUF copies while the collective is in flight.

Residual stream is feature-major: x[128 part, 4 chunks, 512 tok]. MoE is
computed densely (all 8 experts, all tokens); faithful top-2 `combine`
weights (renormalized softmax over the top-2 router logits, 0 elsewhere) are
applied per expert on its token-major output, which is then transposed back
into the feature-major stream with PE transposes.

Precision: routing margins go down to ~3e-6 on router logits and the graded
`idxs` are integer top-2 indices, so every matmul that feeds a router
(everything through moe1's attention + router) runs in true fp32. Compute
strictly after the last router (moe1 experts, tfb1, final LN) runs in
float32r (fp32 storage, ~1e-4 matmul rel-err, full PE rate), which only
perturbs the final logits at ~1e-5.

The classifier tail (mean over each batch's 1024 tokens then
[4,512]@[512,10]) is finished on host from per-core partial column sums
(~41k FLOPs).
"""

import numpy as np

import concourse.bass as bass
import concourse.mybir as mybir
import concourse.tile as tile
from concourse import bacc
from concourse.bass_utils import run_bass_kernel_spmd
from concourse.masks import make_identity

F32 = mybir.dt.float32
F32R = mybir.dt.float32r
U32 = mybir.dt.uint32
AL = mybir.AluOpType
AF = mybir.ActivationFunctionType

P = 128
D = 512
CH = D // P            # 4 feature chunks
NH = 8
DH = 64
HEXP = 2048
HCH = HEXP // P        # 16 hidden chunks
NEXP = 8
TOK = 512              # tokens per core
TCH = TOK // P         # 4 token chunks
EPS = 1e-5
NCORE = 8
B, M, T = 4, 4, 256
S = M * T

PAIR_GROUPS = [[0, 1], [2, 3], [4, 5], [6, 7]]

# post-router-1 matmul dtype. F32R = fast (~1e-4 rel err); F32 = exact.
DT_POST = F32R


def build_program():
    nc = bacc.Bacc("TRN2", target_bir_lowering=False, debug=False,
                   num_devices=NCORE)
    t = {}

    def din(name, shape, dtype=F32):
        t[name] = nc.dram_tensor(name, list(shape), dtype, kind="ExternalInput")

    din("xT0", [D, TOK])
    din("peer_sel", [P, 2])
    for li in range(2):
        dt_moe_exp = F32 if li == 0 else DT_POST
        dt_tfb = F32 if li == 0 else DT_POST
        for fam, dt_att in (("moe", F32), ("tfb", dt_tfb)):
            din(f"{fam}_wqkv_{li}", [D, 3 * D], dt_att)
            din(f"{fam}_bqkv_{li}", [3 * D])
            din(f"{fam}_bqkv_row_{li}", [1, 3 * D], dt_att)
            din(f"{fam}_wo_{li}", [D, D], dt_att)
            din(f"{fam}_bo_{li}", [D])
            din(f"{fam}_ln1_g_{li}", [D]); din(f"{fam}_ln1_b_{li}", [D])
            din(f"{fam}_ln2_g_{li}", [D]); din(f"{fam}_ln2_b_{li}", [D])
        din(f"router_w_{li}", [D, NEXP])
        din(f"router_b_row_{li}", [1, NEXP])
        din(f"moe_w1_{li}", [NEXP, D, HEXP], dt_moe_exp)
        din(f"moe_b1_{li}", [NEXP, HEXP])
        din(f"moe_w2_{li}", [NEXP, HEXP, D], dt_moe_exp)
        din(f"moe_b2_row_{li}", [NEXP, 1, D], dt_moe_exp)
        din(f"tfb_w1_{li}", [D, HEXP], dt_tfb)
        din(f"tfb_b1_{li}", [HEXP])
        din(f"tfb_w2_{li}", [HEXP, D], dt_tfb)
        din(f"tfb_b2_{li}", [D])
    din("fin_g", [D]); din("fin_b", [D])

    gate_out = nc.dram_tensor("gate_out", [2, TCH, P, NEXP], F32,
                              kind="ExternalOutput")
    idx_out = nc.dram_tensor("idx_out", [2, TCH, P, 2], U32,
                             kind="ExternalOutput")
    lnsum_out = nc.dram_tensor("lnsum_out", [P, CH], F32,
                               kind="ExternalOutput")

    with tile.TileContext(nc) as tc:
        _emit(tc, t, gate_out, idx_out, lnsum_out)
    nc.finalize()
    return nc


DEBUG = False
_DBG_TENSORS = {}


def _emit(tc, t, gate_out, idx_out, lnsum_out):
    nc = tc.nc
    pools = []

    def dbg(name, ap):
        if not DEBUG:
            return
        o = nc.dram_tensor(f"dbg_{name}", list(ap.shape), ap.dtype,
                           kind="ExternalOutput")
        _DBG_TENSORS[name] = o
        nc.sync.dma_start(o[:], ap)
    _emit.dbg = dbg

    def persistent_pool(**kw):
        p = tc.tile_pool(**kw)
        pools.append(p)
        return p.__enter__()

    const = persistent_pool(name="const", bufs=1)
    resid = persistent_pool(name="resid", bufs=3)
    dram = persistent_pool(name="dram", bufs=2, space="DRAM")

    ones_row = {}
    ones_col = {}
    orow_f = const.tile([1, TOK], F32, name="ones_row_f")
    ocol_f = const.tile([P, 1], F32, name="ones_col_f")
    nc.vector.memset(orow_f[:], 1.0)
    nc.vector.memset(ocol_f[:], 1.0)
    ones_row[F32] = orow_f
    ones_col[F32] = ocol_f
    if DT_POST != F32:
        orow_r = const.tile([1, TOK], DT_POST, name="ones_row_r")
        ocol_r = const.tile([P, 1], DT_POST, name="ones_col_r")
        nc.vector.tensor_copy(orow_r[:], orow_f[:])
        nc.vector.tensor_copy(ocol_r[:], ocol_f[:])
        ones_row[DT_POST] = orow_r
        ones_col[DT_POST] = ocol_r
    ones_vex = const.tile([P, TCH, NH, 1], F32, name="ones_vex")
    nc.vector.memset(ones_vex[:], 1.0)
    psel = const.tile([P, 2], F32, name="psel")
    nc.sync.dma_start(psel[:], t["peer_sel"][:])
    eps1 = const.tile([1, 1], F32, name="eps1")
    nc.vector.memset(eps1[:], EPS)
    ident = const.tile([P, P], F32, name="ident")
    make_identity(nc, ident[:])

    def col_pc(name):
        tl = const.tile([P, CH], F32, name=f"c_{name}")
        nc.sync.dma_start(tl[:], t[name].rearrange("(c p) -> p c", p=P))
        return tl

    lnp = {}
    for li in range(2):
        for k in ("moe_ln1", "moe_ln2", "tfb_ln1", "tfb_ln2"):
            lnp[(k, li)] = (col_pc(f"{k}_g_{li}"), col_pc(f"{k}_b_{li}"))
    fin_gt, fin_bt = col_pc("fin_g"), col_pc("fin_b")

    x = resid.tile([P, CH, TOK], F32, tag="xres", name="x0")
    nc.sync.dma_start(x[:], t["xT0"].rearrange("(c p) t -> p c t", p=P))

    # ---------------------------------------------------------------- LN ----
    def layernorm(x_in, gt, bt, out_dt, scope):
        oc_in = ones_col[x_in.dtype]
        with (
            tc.tile_pool(name=f"{scope}sb", bufs=1) as sp,
            tc.tile_pool(name=f"{scope}ps", bufs=1, space="PSUM") as pp,
        ):
            s1 = pp.tile([1, TOK], F32, tag="s1", name="s1")
            s2 = pp.tile([1, TOK], F32, tag="s2", name="s2")
            sq = sp.tile([P, TOK], F32, tag="sq", name="sq")
            for c in range(CH):
                nc.tensor.matmul(s1[:], oc_in[:], x_in[:, c, :],
                                 start=(c == 0), stop=(c == CH - 1))
            for c in range(CH):
                nc.vector.tensor_tensor(sq[:], x_in[:, c, :], x_in[:, c, :],
                                        AL.mult)
                nc.tensor.matmul(s2[:], ones_col[F32], sq[:],
                                 start=(c == 0), stop=(c == CH - 1))
            rows = sp.tile([1, 2, TOK], F32, tag="rows", name="rows")
            mean = rows[:, 0, :]
            rstd = rows[:, 1, :]
            nc.vector.tensor_scalar_mul(mean, s1[:], 1.0 / D)
            nc.vector.tensor_tensor(rstd, mean, mean, AL.mult)
            # var = s2/D - mean^2
            nc.vector.scalar_tensor_tensor(rstd, s2[:], 1.0 / D, rstd,
                                           AL.mult, AL.subtract)
            nc.scalar.activation(rstd, rstd, AF.Sqrt, bias=eps1[:])
            nc.vector.reciprocal(rstd, rstd)
            bc1 = pp.tile([P, TOK], F32, tag="bc1", name="bc1")
            bc2 = pp.tile([P, TOK], F32, tag="bc2", name="bc2")
            nc.tensor.matmul(bc1[:], ones_row[F32][0:1, 0:P], rows[:, 0, :],
                             start=True, stop=True)
            nc.tensor.matmul(bc2[:], ones_row[F32][0:1, 0:P], rows[:, 1, :],
                             start=True, stop=True)
            mb = sp.tile([P, TOK], F32, tag="mb", name="mb")
            rb = sp.tile([P, TOK], F32, tag="rb", name="rb")
            nc.scalar.copy(mb[:], bc1[:])
            nc.scalar.copy(rb[:], bc2[:])
            out = resid.tile([P, CH, TOK], out_dt, tag="xres",
                             name=f"{scope}out")
            tmp = sp.tile([P, TOK], F32, tag="tmp", name="tmp")
            for c in range(CH):
                nc.vector.tensor_tensor(tmp[:], x_in[:, c, :], mb[:],
                                        AL.subtract)
                nc.vector.tensor_tensor(tmp[:], tmp[:], rb[:], AL.mult)
                nc.vector.tensor_scalar(out[:, c, :], tmp[:], gt[:, c:c + 1],
                                        bt[:, c:c + 1], AL.mult, AL.add)
        return out

    # --------------------------------------------------------- attention ----
    def attention(x_in, fam, li, dt, scope, safe_softmax=False):
        """Returns xpre = x_in + attn_out + bias (fp32, feature-major).

        safe_softmax: subtract the exact per-query max before exp (needed only
        for attn0, whose input is the unnormalized residual stream where raw
        scores reach +-500; later attentions see LN'd inputs, |s/8| < 2)."""
        orow = ones_row[dt]
        with (
            tc.tile_pool(name=f"{scope}w", bufs=1) as wp,
            tc.tile_pool(name=f"{scope}sb", bufs=1) as sp,
            tc.tile_pool(name=f"{scope}oh", bufs=1) as ohp,
            tc.tile_pool(name=f"{scope}ps", bufs=2, space="PSUM") as pp,
            tc.tile_pool(name=f"{scope}po", bufs=2, space="PSUM") as ppo,
        ):
            wqkv = wp.tile([P, CH, 3 * D], dt, name="wqkv")
            nc.sync.dma_start(
                wqkv[:], t[f"{fam}_wqkv_{li}"].rearrange("(c p) o -> p c o", p=P))
            bqkv_col = wp.tile([P, 12], F32, name="bqkv_col")
            nc.sync.dma_start(
                bqkv_col[:], t[f"{fam}_bqkv_{li}"].rearrange("(o p) -> p o", p=P))
            bqkv_row = wp.tile([1, 3 * D], dt, name="bqkv_row")
            nc.sync.dma_start(bqkv_row[:], t[f"{fam}_bqkv_row_{li}"][:])
            # head-major layout: head h's 64 contraction rows at base 0
            wo_sb = wp.tile([DH, NH, D], dt, name="wo_sb")
            nc.sync.dma_start(
                wo_sb[:], t[f"{fam}_wo_{li}"].rearrange("(h p) o -> p h o", p=DH))
            bo_col = wp.tile([P, CH], F32, name="bo_col")
            nc.sync.dma_start(
                bo_col[:], t[f"{fam}_bo_{li}"].rearrange("(o p) -> p o", p=P))

            # --- Q, K feature-major; V token-major ---
            qT = sp.tile([P, CH, TOK], dt, tag="qT", name="qT")
            kT = sp.tile([P, CH, TOK], dt, tag="kT", name="kT")
            for oc in range(2 * CH):          # 0..3 -> Q, 4..7 -> K
                ps = pp.tile([P, TOK], F32, tag="mm512", name=f"qk{oc}")
                for kc in range(CH):
                    nc.tensor.matmul(ps[:], wqkv[:, kc, oc * P:(oc + 1) * P],
                                     x_in[:, kc, :], start=(kc == 0),
                                     stop=(kc == CH - 1))
                dst = qT if oc < CH else kT
                nc.scalar.activation(dst[:, oc % CH, :], ps[:], AF.Identity,
                                     bias=bqkv_col[:, oc:oc + 1])
            v_sb = sp.tile([P, TCH, D], dt, tag="v_sb", name="v_sb")
            for j in range(TCH):
                ps = pp.tile([P, D], F32, tag="mm512", name=f"v{j}")
                for kc in range(CH):
                    nc.tensor.matmul(ps[:], x_in[:, kc, j * P:(j + 1) * P],
                                     wqkv[:, kc, 2 * D:3 * D],
                                     start=(kc == 0), stop=False)
                nc.tensor.matmul(ps[:], orow[0:1, 0:P],
                                 bqkv_row[:, 2 * D:3 * D], start=False,
                                 stop=True)
                nc.scalar.copy(v_sb[:, j, :], ps[:])

            # --- pair KV exchange (overlapped with own-half attention) ---
            kv_send = dram.tile([2 * D, TOK], dt, tag="kv_send", name="kv_send")
            kv_recv = dram.tile([2, 2 * D, TOK], dt, tag="kv_recv",
                                name="kv_recv")
            nc.sync.dma_start(
                kv_send[0:D].rearrange("(c p) t -> p c t", p=P), kT[:])
            nc.sync.dma_start(
                kv_send[D:2 * D].rearrange("(c p) t -> p c t", p=P), v_sb[:])
            nc.gpsimd.collective_compute(
                "AllGather", AL.bypass, replica_groups=PAIR_GROUPS,
                ins=[kv_send.opt()], outs=[kv_recv.opt()])
            raw = []
            for s in range(2):
                kr = sp.tile([P, CH, TOK], dt, tag=f"kr{s}", name=f"kr{s}")
                vr = sp.tile([P, TCH, D], dt, tag=f"vr{s}", name=f"vr{s}")
                nc.sync.dma_start(
                    kr[:], kv_recv[s, 0:D].rearrange("(c p) t -> p c t", p=P))
                nc.sync.dma_start(
                    vr[:],
                    kv_recv[s, D:2 * D].rearrange("(c p) t -> p c t", p=P))
                raw.append((kr, vr))
            pkT, pv = raw[0]
            for dst, a1 in ((pkT, raw[1][0]), (pv, raw[1][1])):
                nc.vector.tensor_scalar_mul(dst[:], dst[:], psel[:, 0:1])
                nc.vector.scalar_tensor_tensor(dst[:], a1[:], psel[:, 1:2],
                                               dst[:], AL.mult, AL.add)

            # --- [V | 1] stacks for fused AV+rowsum ---
            vex = []
            for nm, src in (("own", v_sb), ("peer", pv)):
                ve = sp.tile([P, TCH, NH, DH + 1], dt, tag=f"vex{nm}",
                             name=f"vex{nm}")
                nc.vector.tensor_copy(
                    ve[:, :, :, 0:DH],
                    src[:].rearrange("p j (h d) -> p j h d", h=NH))
                nc.vector.tensor_copy(ve[:, :, :, DH:DH + 1], ones_vex[:])
                vex.append(ve)

            # --- per head: scores -> exp -> AV(+rowsum) -> normalize ---
            oh = [ohp.tile([DH, TOK], dt, name=f"oh{h}") for h in range(NH)]
            for h in range(NH):
                hp = (h % 2) * DH
                hc = h // 2
                mb_h = None
                if safe_softmax:
                    # exact per-query max over all 1024 keys, via q-major
                    # score matmuls; broadcast it over the k partitions
                    m_row = sp.tile([1, TOK], F32, tag="m_row", name="m_row",
                                    bufs=2)
                    for qc in range(TCH):
                        mpart = sp.tile([P, 2], F32, tag="mpart", name="mpart",
                                        bufs=2)
                        for src_i, src_k in ((0, kT), (1, pkT)):
                            ps_qm = pp.tile([P, TOK], F32, tag="mm512",
                                            name=f"qm{h}_{qc}_{src_i}")
                            nc.tensor.matmul(
                                ps_qm[:], qT[hp:hp + DH, hc, qc * P:(qc + 1) * P],
                                src_k[hp:hp + DH, hc, :], start=True, stop=True)
                            nc.vector.tensor_reduce(
                                mpart[:, src_i:src_i + 1], ps_qm[:],
                                mybir.AxisListType.X, AL.max)
                        mq = sp.tile([P, 1], F32, tag="mq", name="mq", bufs=2)
                        nc.vector.tensor_reduce(mq[:], mpart[:],
                                                mybir.AxisListType.X, AL.max)
                        ps_tr = pp.tile([1, P], F32, tag="ps_tr",
                                        name=f"tr{h}_{qc}")
                        nc.tensor.transpose(ps_tr[:], mq[:], ident[:])
                        nc.scalar.copy(m_row[:, qc * P:(qc + 1) * P], ps_tr[:])
                    ps_mb = pp.tile([P, TOK], F32, tag="mm512",
                                    name=f"mb{h}")
                    nc.tensor.matmul(ps_mb[:], ones_row[F32][0:1, 0:P],
                                     m_row[:], start=True, stop=True)
                    mb_h = sp.tile([P, TOK], F32, tag="mb_h", name="mb_h",
                                   bufs=2)
                    nc.scalar.copy(mb_h[:], ps_mb[:])
                po = ppo.tile([DH + 1, TOK], F32, tag="po", name=f"po{h}")
                nmm = 0
                for src_i, src_k in ((0, kT), (1, pkT)):
                    for j in range(TCH):
                        ps_s = pp.tile([P, TOK], F32, tag="mm512",
                                       name=f"s{h}_{src_i}{j}")
                        nc.tensor.matmul(
                            ps_s[:], src_k[hp:hp + DH, hc, j * P:(j + 1) * P],
                            qT[hp:hp + DH, hc, :], start=True, stop=True)
                        ae = sp.tile([P, TOK], dt, tag="ae", name="ae", bufs=3)
                        if safe_softmax:
                            sdiff = sp.tile([P, TOK], F32, tag="sdiff",
                                            name="sdiff", bufs=2)
                            nc.vector.tensor_tensor(sdiff[:], ps_s[:], mb_h[:],
                                                    AL.subtract)
                            nc.scalar.activation(ae[:], sdiff[:], AF.Exp,
                                                 scale=0.125)
                        else:
                            nc.scalar.activation(ae[:], ps_s[:], AF.Exp,
                                                 scale=0.125)
                        nc.tensor.matmul(po[:], vex[src_i][:, j, h, :], ae[:],
                                         start=(nmm == 0), stop=(nmm == 7))
                        nmm += 1
                rsal = sp.tile([DH + 1, TOK], F32, tag="rsal", name="rsal")
                nc.vector.reciprocal(rsal[DH:DH + 1, :], po[DH:DH + 1, :])
                rs0 = sp.tile([1, TOK], F32, tag="rs0", name="rs0")
                nc.sync.dma_start(rs0[:], rsal[DH:DH + 1, :])
                ps_rb = pp.tile([DH, TOK], F32, tag="ps_rb", name=f"rb{h}")
                nc.tensor.matmul(ps_rb[:], ones_row[F32][0:1, 0:DH], rs0[:],
                                 start=True, stop=True)
                rbs = sp.tile([DH, TOK], F32, tag="rbs", name="rbs")
                nc.scalar.copy(rbs[:], ps_rb[:])
                nc.vector.tensor_tensor(oh[h][:], po[0:DH, :], rbs[:], AL.mult)

            # --- wo projection + bias + residual ---
            xpre = resid.tile([P, CH, TOK], F32, tag="xres",
                              name=f"{scope}xpre")
            for dc in range(CH):
                ps = pp.tile([P, TOK], F32, tag="mm512", name=f"wo{dc}")
                for h in range(NH):
                    nc.tensor.matmul(
                        ps[:], wo_sb[:, h, dc * P:(dc + 1) * P],
                        oh[h][:], start=(h == 0), stop=(h == NH - 1))
                nc.vector.scalar_tensor_tensor(xpre[:, dc, :], ps[:],
                                               bo_col[:, dc:dc + 1],
                                               x_in[:, dc, :], AL.add, AL.add)
        return xpre

    # --------------------------------------------------------------- MoE ----
    def moe(x_ln, li, scope):
        dt = F32 if li == 0 else DT_POST
        with (
            tc.tile_pool(name=f"{scope}w", bufs=2) as wp,
            tc.tile_pool(name=f"{scope}w2", bufs=1) as wp2,
            tc.tile_pool(name=f"{scope}sb", bufs=1) as sp,
            tc.tile_pool(name=f"{scope}h", bufs=1) as hp_,
            tc.tile_pool(name=f"{scope}ps", bufs=2, space="PSUM") as pp,
        ):
            # ---- router + gates + top-2 combine (always fp32) ----
            rw = wp2.tile([P, CH, NEXP], F32, name="rw")
            nc.sync.dma_start(
                rw[:], t[f"router_w_{li}"].rearrange("(c p) e -> p c e", p=P))
            rb_row = wp2.tile([1, NEXP], F32, name="rb_row")
            nc.sync.dma_start(rb_row[:], t[f"router_b_row_{li}"][:])
            cmb = sp.tile([P, TCH, NEXP], F32, tag="cmb", name="cmb")
            for tcx in range(TCH):
                ps_l = pp.tile([P, NEXP], F32, tag="ps_l", name=f"lg{tcx}")
                for fc in range(CH):
                    nc.tensor.matmul(ps_l[:],
                                     x_ln[:, fc, tcx * P:(tcx + 1) * P],
                                     rw[:, fc, :], start=(fc == 0), stop=False)
                nc.tensor.matmul(ps_l[:], ones_row[F32][0:1, 0:P], rb_row[:],
                                 start=False, stop=True)
                lg = sp.tile([P, NEXP], F32, tag="lg", name="lg")
                nc.scalar.copy(lg[:], ps_l[:])
                mx8 = sp.tile([P, NEXP], F32, tag="mx8", name="mx8")
                ix8 = sp.tile([P, NEXP], U32, tag="ix8", name="ix8")
                nc.vector.max(out=mx8[:], in_=lg[:])
                nc.vector.max_index(out=ix8[:], in_max=mx8[:], in_values=lg[:])
                nc.sync.dma_start(idx_out[li, tcx], ix8[:, 0:2])
                nm1 = sp.tile([P, 1], F32, tag="nm1", name="nm1")
                nc.vector.tensor_scalar_mul(nm1[:], mx8[:, 0:1], -1.0)
                ge = sp.tile([P, NEXP], F32, tag="ge", name="ge")
                gs = sp.tile([P, 1], F32, tag="gs", name="gs")
                nc.scalar.activation(ge[:], lg[:], AF.Exp, bias=nm1[:],
                                     accum_out=gs[:])
                nc.vector.reciprocal(gs[:], gs[:])
                nc.vector.tensor_scalar_mul(ge[:], ge[:], gs[:, 0:1])
                nc.sync.dma_start(gate_out[li, tcx], ge[:])
                # p1 = 1/(1+e2), p2 = e2*p1, e2 = exp(m2 - m1)
                e2 = sp.tile([P, 1], F32, tag="e2", name="e2")
                nc.vector.tensor_tensor(e2[:], mx8[:, 1:2], mx8[:, 0:1],
                                        AL.subtract)
                nc.scalar.activation(e2[:], e2[:], AF.Exp)
                p1 = sp.tile([P, 1], F32, tag="p1", name="p1")
                nc.vector.tensor_scalar_add(p1[:], e2[:], 1.0)
                nc.vector.reciprocal(p1[:], p1[:])
                p2 = sp.tile([P, 1], F32, tag="p2", name="p2")
                nc.vector.tensor_tensor(p2[:], e2[:], p1[:], AL.mult)
                eq1 = sp.tile([P, NEXP], F32, tag="eq1", name="eq1")
                eq2 = sp.tile([P, NEXP], F32, tag="eq2", name="eq2")
                nc.vector.tensor_scalar(eq1[:], lg[:], mx8[:, 0:1], p1[:, 0:1],
                                        AL.is_equal, AL.mult)
                nc.vector.tensor_scalar(eq2[:], lg[:], mx8[:, 1:2], p2[:, 0:1],
                                        AL.is_equal, AL.mult)
                nc.vector.tensor_tensor(cmb[:, tcx, :], eq1[:], eq2[:], AL.add)

            # ---- dense experts ----
            if dt != x_ln.dtype:
                xe = sp.tile([P, CH, TOK], dt, tag="xe", name="xe")
                nc.vector.tensor_copy(xe[:], x_ln[:])
            else:
                xe = x_ln
            mo = sp.tile([P, TCH, D], F32, tag="mo", name="mo")  # token-major
            for e in range(NEXP):
                w1 = wp.tile([P, CH, HEXP], dt, tag="w1", name=f"w1_{e}")
                nc.sync.dma_start(
                    w1[:],
                    t[f"moe_w1_{li}"][e].rearrange("(c p) hh -> p c hh", p=P))
                b1c = wp.tile([P, HCH], F32, tag="b1c", name=f"b1c_{e}")
                nc.sync.dma_start(
                    b1c[:], t[f"moe_b1_{li}"][e].rearrange("(hc p) -> p hc", p=P))
                w2 = wp2.tile([P, HCH, D], dt, tag="w2", name=f"w2_{e}")
                nc.sync.dma_start(
                    w2[:],
                    t[f"moe_w2_{li}"][e].rearrange("(hc p) d -> p hc d", p=P))
                b2r = wp.tile([1, D], dt, tag="b2r", name=f"b2r_{e}")
                nc.sync.dma_start(b2r[:], t[f"moe_b2_row_{li}"][e])
                hT = hp_.tile([P, HCH, TOK], dt, tag="hT", name=f"hT_{e}")
                for hc in range(HCH):
                    ps_h = pp.tile([P, TOK], F32, tag="ps_h",
                                   name=f"h{e}_{hc}")
                    for kc in range(CH):
                        nc.tensor.matmul(ps_h[:],
                                         w1[:, kc, hc * P:(hc + 1) * P],
                                         xe[:, kc, :], start=(kc == 0),
                                         stop=(kc == CH - 1))
                    nc.scalar.activation(hT[:, hc, :], ps_h[:], AF.Relu,
                                         bias=b1c[:, hc:hc + 1])
                for tcx in range(TCH):
                    ps_eo = pp.tile([P, D], F32, tag="ps_eo",
                                    name=f"eo{e}_{tcx}")
                    for hc in range(HCH):
                        nc.tensor.matmul(ps_eo[:],
                                         hT[:, hc, tcx * P:(tcx + 1) * P],
                                         w2[:, hc, :], start=(hc == 0),
                                         stop=False)
                    nc.tensor.matmul(ps_eo[:], ones_row[dt][0:1, 0:P], b2r[:],
                                     start=False, stop=True)
                    if e == 0:
                        nc.vector.tensor_scalar(mo[:, tcx, :], ps_eo[:],
                                                cmb[:, tcx, 0:1], None,
                                                AL.mult)
                    else:
                        nc.vector.scalar_tensor_tensor(
                            mo[:, tcx, :], ps_eo[:], cmb[:, tcx, e:e + 1],
                            mo[:, tcx, :], AL.mult, AL.add)
            # ---- transpose token-major mo back to feature-major + residual --
            xpre = resid.tile([P, CH, TOK], F32, tag="xres",
                              name=f"{scope}xpre")
            for tcx in range(TCH):
                for c in range(CH):
                    ps_t = pp.tile([P, P], F32, tag="ps_t",
                                   name=f"t{tcx}_{c}")
                    nc.tensor.transpose(ps_t[:], mo[:, tcx, c * P:(c + 1) * P],
                                        ident[:])
                    nc.vector.tensor_tensor(
                        xpre[:, c, tcx * P:(tcx + 1) * P], ps_t[:],
                        x_ln[:, c, tcx * P:(tcx + 1) * P], AL.add)
        return xpre

    # --------------------------------------------------------------- FFN ----
    def ffn(x_ln, li, scope):
        dt = F32 if li == 0 else DT_POST
        with (
            tc.tile_pool(name=f"{scope}w", bufs=1) as wp,
            tc.tile_pool(name=f"{scope}sb", bufs=1) as sp,
            tc.tile_pool(name=f"{scope}ps", bufs=2, space="PSUM") as pp,
        ):
            w1 = wp.tile([P, CH, HEXP], dt, name="w1")
            nc.sync.dma_start(
                w1[:], t[f"tfb_w1_{li}"].rearrange("(c p) hh -> p c hh", p=P))
            b1c = wp.tile([P, HCH], F32, name="b1c")
            nc.sync.dma_start(
                b1c[:], t[f"tfb_b1_{li}"].rearrange("(hc p) -> p hc", p=P))
            w2 = wp.tile([P, HCH, D], dt, name="w2")
            nc.sync.dma_start(
                w2[:], t[f"tfb_w2_{li}"].rearrange("(hc p) d -> p hc d", p=P))
            b2c = wp.tile([P, CH], F32, name="b2c")
            nc.sync.dma_start(
                b2c[:], t[f"tfb_b2_{li}"].rearrange("(c p) -> p c", p=P))
            if dt != x_ln.dtype:
                xe = sp.tile([P, CH, TOK], dt, tag="xe", name="xe")
                nc.vector.tensor_copy(xe[:], x_ln[:])
            else:
                xe = x_ln
            hT = sp.tile([P, HCH, TOK], dt, tag="hT", name="hT")
            for hc in range(HCH):
                ps_h = pp.tile([P, TOK], F32, tag="ps_h", name=f"h{hc}")
                for kc in range(CH):
                    nc.tensor.matmul(ps_h[:], w1[:, kc, hc * P:(hc + 1) * P],
                                     xe[:, kc, :], start=(kc == 0),
                                     stop=(kc == CH - 1))
                nc.scalar.activation(hT[:, hc, :], ps_h[:], AF.Relu,
                                     bias=b1c[:, hc:hc + 1])
            xpre = resid.tile([P, CH, TOK], F32, tag="xres",
                              name=f"{scope}xpre")
            for dc in range(CH):
                ps_f = pp.tile([P, TOK], F32, tag="ps_f", name=f"f{dc}")
                for hc in range(HCH):
                    nc.tensor.matmul(ps_f[:], w2[:, hc, dc * P:(dc + 1) * P],
                                     hT[:, hc, :], start=(hc == 0),
                                     stop=(hc == HCH - 1))
                nc.vector.scalar_tensor_tensor(xpre[:, dc, :], ps_f[:],
                                               b2c[:, dc:dc + 1],
                                               x_ln[:, dc, :], AL.add, AL.add)
        return xpre

    # ------------------------------------------------------------- model ----
    dbg("x0", x[:])
    for li in range(2):
        dt_tfb = F32 if li == 0 else DT_POST
        xa = attention(x, "moe", li, F32, f"A{li}", safe_softmax=(li == 0))
        dbg(f"xa{li}", xa[:])
        x = layernorm(xa, *lnp[("moe_ln1", li)], F32, f"LA{li}")
        dbg(f"xln{li}", x[:])
        xm = moe(x, li, f"M{li}")
        dbg(f"xm{li}", xm[:])
        x = layernorm(xm, *lnp[("moe_ln2", li)],
                      F32 if li == 0 else DT_POST, f"LM{li}")
        xa = attention(x, "tfb", li, dt_tfb, f"B{li}")
        dbg(f"xb{li}", xa[:])
        x = layernorm(xa, *lnp[("tfb_ln1", li)], dt_tfb, f"LB{li}")
        xf = ffn(x, li, f"F{li}")
        dbg(f"xf{li}", xf[:])
        x = layernorm(xf, *lnp[("tfb_ln2", li)],
                      F32 if li == 0 else DT_POST, f"LF{li}")

    xfin = layernorm(x, fin_gt, fin_bt, F32, "LFIN")
    with tc.tile_pool(name="finsb", bufs=1) as sp:
        csum = sp.tile([P, CH], F32, name="csum")
        nc.vector.tensor_reduce(csum[:], xfin[:], mybir.AxisListType.X, AL.add)
        nc.sync.dma_start(lnsum_out[:], csum[:])

    for pl in reversed(pools):
        pl.__exit__(None, None, None)


# ======================================================================
# host side
# ======================================================================
_PROG = None
LAST_EXEC_NS = None
LAST_RESULTS = None


def _pos_encoding():
    pos = np.arange(S, dtype=np.float32)[:, None]
    div = np.exp(np.arange(0, D, 2, dtype=np.float32)
                 * np.float32(-np.log(10000.0) / D))
    pe = np.zeros((S, D), np.float32)
    pe[:, 0::2] = np.sin(pos * div)
    pe[:, 1::2] = np.cos(pos * div)
    return pe


def kernel(**inputs):
    global _PROG, LAST_EXEC_NS
    import time

    if _PROG is None:
        _PROG = build_program()
    nc = _PROG

    f32 = np.float32

    def arr(x):
        return np.ascontiguousarray(np.asarray(x, f32))

    emb = arr(inputs["token_embeddings"]).reshape(B, S, D)
    x0 = emb * np.sqrt(np.float32(D)) + _pos_encoding()[None]

    shared = {}
    for li in range(2):
        for fam, src in (("moe", "moe_attn"), ("tfb", "tfb_attn")):
            shared[f"{fam}_wqkv_{li}"] = arr(inputs[f"{src}_wqkv"][li])
            shared[f"{fam}_bqkv_{li}"] = arr(inputs[f"{src}_bqkv"][li])
            shared[f"{fam}_bqkv_row_{li}"] = shared[f"{fam}_bqkv_{li}"][None, :]
            shared[f"{fam}_wo_{li}"] = arr(inputs[f"{src}_wo"][li])
            shared[f"{fam}_bo_{li}"] = arr(inputs[f"{src}_bo"][li])
            for suf in ("ln1_g", "ln1_b", "ln2_g", "ln2_b"):
                shared[f"{fam}_{suf}_{li}"] = arr(inputs[f"{fam}_{suf}"][li])
        shared[f"router_w_{li}"] = arr(inputs["moe_router_w"][li])
        shared[f"router_b_row_{li}"] = arr(inputs["moe_router_b"][li])[None, :]
        shared[f"moe_w1_{li}"] = arr(inputs["moe_w1"][li])
        shared[f"moe_b1_{li}"] = arr(inputs["moe_b1"][li])
        shared[f"moe_w2_{li}"] = arr(inputs["moe_w2"][li])
        shared[f"moe_b2_row_{li}"] = arr(inputs["moe_b2"][li])[:, None, :]
        shared[f"tfb_w1_{li}"] = arr(inputs["tfb_w1"][li])
        shared[f"tfb_b1_{li}"] = arr(inputs["tfb_b1"][li])
        shared[f"tfb_w2_{li}"] = arr(inputs["tfb_w2"][li])
        shared[f"tfb_b2_{li}"] = arr(inputs["tfb_b2"][li])
    shared["fin_g"] = arr(inputs["final_ln_g"])
    shared["fin_b"] = arr(inputs["final_ln_b"])

    in_maps = []
    for c in range(NCORE):
        b, h = c // 2, c % 2
        m = dict(shared)
        m["xT0"] = np.ascontiguousarray(x0[b, h * TOK:(h + 1) * TOK, :].T)
        sel = np.zeros((P, 2), f32)
        sel[:, 1 - h] = 1.0         # even core's peer is slot 1, odd's slot 0
        m["peer_sel"] = sel
        in_maps.append(m)

    t0 = time.time()
    r = run_bass_kernel_spmd(nc, in_maps, core_ids=list(range(NCORE)))
    LAST_EXEC_NS = int((time.time() - t0) * 1e9)
    global LAST_RESULTS
    LAST_RESULTS = r.results

    gates = np.zeros((2, B, S, NEXP), f32)
    idxs = np.zeros((2, B, S, 2), np.int32)
    means = np.zeros((B, D), f32)
    for c in range(NCORE):
        b, h = c // 2, c % 2
        res = r.results[c]
        gates[:, b, h * TOK:(h + 1) * TOK, :] = \
            res["gate_out"].reshape(2, TOK, NEXP)
        idxs[:, b, h * TOK:(h + 1) * TOK, :] = \
            res["idx_out"].reshape(2, TOK, 2).astype(np.int32)
        means[b] += res["lnsum_out"].T.reshape(D)
    means /= np.float32(S)
    logits = means @ np.asarray(inputs["out_w"], f32) \
        + np.asarray(inputs["out_b"], f32)
    return logits, gates, idxs
